# revision 7
# baseline (speedup 1.0000x reference)
"""Deformable-correlation-fixed-weight kernel for 8 TRN2 NeuronCores.

Math: out[b, t*K+k, h, w] = sum_c samp[b,c,k,h,w] * weight[c,t,k].
With weight constant along c (DefCorFixW: weight = 1/C), this equals
s[t,k] * bilinear(mean_c x[b], py[b,k], px[b,k]); the device computes
the channel-mean image and the 9 bilinear-sampled maps per batch; the
host replicates over t and scales by s[t,k] = sum_c weight[c,t,k].

Sharding: data-parallel over batch B=8 across the 8 cores.

v3 design (pipeline-fill + preamble rework over v2):
  Pixels n = h*96+w are laid out n = 72*p + i (p partition, i slot).
  Offsets clamped to +-3.9995 => a 9x9 hat window suffices.

  Changes vs v2:
   - mean scale 1/C folded into the matmul `ones` table; ACT does 9
     1024-wide PSUM->SBUF copies instead of 18 512-wide scaled ones.
   - impad written in 6 sixths, strip fetched in 4 quarter-DMAs on
     4 different queues, each gated on exactly the impad prefix it
     needs -> first tap starts ~12us instead of ~34us.
   - dX computed in ONE fp16 2x-mode tensor_tensor per kx group in a
     [t, j, s] layout against a host delta table (replaces f32 px0 +
     3 broadcast-limited 1x subtracts); |dX| on ACT in the same
     layout; the Relu(1-|x|) ACT op writes wX transposed back to
     [s, j] for the window product.
   - dY/|dY| never materialized on DVE: 9 ACT Abs ops with bias 4-j
     read oy_cl directly; one big Relu makes wY.
   - res/out in bf16 (host upcasts); output DMA halves.
"""

import numpy as np

B, C, H, W = 8, 128, 96, 96
K = 9
T = 9
HW = H * W
P = 128          # partitions
S = HW // P      # 72 pixels per partition
AW = 9           # window side (rows and cols)
CLAMP = 3.9995
PADR = 8         # zero rows above/below in the flat padded image
NPAD = (H + 2 * PADR) * W          # 10752
STRIPLEN = 1042                    # per-partition strip (flat span)
STRIPOFF = 283                     # 72p - 485 + PADR*96
NCH = 512
NCHUNK = HW // NCH                 # 18
XCHUNKS = (1, 2, 3, 3, 3, 3, 3)
QMAP = (0, 1, 2, 1, 2, 1, 0)
# impad sixth s may be written after ACT copy SIXTH_GATE[s] (1-based)
SIXTH_GATE = (2, 3, 5, 6, 8, 9)
# strip quarter q needs sC >= STRIP_GATE[q] (zt,zb,s0..s5 in order, 16 each)
STRIP_GATE = (64, 96, 112, 128)

_cached = {}


def _build_nc():
    import concourse.bass as bass
    import concourse.mybir as mybir
    from contextlib import ExitStack

    f32 = mybir.dt.float32
    bf16 = mybir.dt.bfloat16
    fp16 = mybir.dt.float16
    Alu = mybir.AluOpType
    Act = mybir.ActivationFunctionType

    nc = bass.Bass(detect_race_conditions=False)

    x_ext = nc.declare_dram_parameter("x", [C, HW], bf16, isOutput=False)
    off_ext = nc.declare_dram_parameter("offset", [P, 2 * K * S], fp16,
                                        isOutput=False)
    delta_ext = nc.declare_dram_parameter("delta2", [P, 3 * AW * S], fp16,
                                          isOutput=False)
    ones_ext = nc.declare_dram_parameter("ones", [C, 2], bf16, isOutput=False)
    biasy_ext = nc.declare_dram_parameter("biasy", [P, AW], f32, isOutput=False)
    out_ext = nc.declare_dram_parameter("out", [P, K * S], bf16, isOutput=True)

    impad = nc.dram_tensor("impad", [NPAD], bf16)

    with ExitStack() as ctx:
        xb = ctx.enter_context(nc.sbuf_tensor([C, HW], bf16))
        off_sb = ctx.enter_context(nc.sbuf_tensor([P, 2 * K, S], fp16))
        delta_sb = ctx.enter_context(nc.sbuf_tensor([P, 3, AW, S], fp16))
        ones_sb = ctx.enter_context(nc.sbuf_tensor([C, 2], bf16))
        biasy_sb = ctx.enter_context(nc.sbuf_tensor([P, AW], f32))
        m_flat = ctx.enter_context(nc.sbuf_tensor([1, HW], bf16))
        zt = ctx.enter_context(nc.sbuf_tensor([1, PADR * W], bf16))
        strip = ctx.enter_context(nc.sbuf_tensor([P, STRIPLEN], bf16))
        ox_cl = ctx.enter_context(nc.sbuf_tensor([P, K, S], fp16))
        oy_cl = ctx.enter_context(nc.sbuf_tensor([P, K, S], fp16))
        dXt = ctx.enter_context(nc.sbuf_tensor([P, 3, 3, AW, S], fp16))
        aXt = ctx.enter_context(nc.sbuf_tensor([P, 3, 3, AW, S], fp16))
        absY = ctx.enter_context(nc.sbuf_tensor([P, K, S, AW], fp16))
        wX = ctx.enter_context(nc.sbuf_tensor([P, K, S, AW], bf16))
        wY = ctx.enter_context(nc.sbuf_tensor([P, K, S, AW], bf16))
        prod = ctx.enter_context(nc.sbuf_tensor([P, 2, S, AW, AW], bf16))
        t4 = ctx.enter_context(nc.sbuf_tensor([P, 2, S, AW, 4], bf16))
        t2 = ctx.enter_context(nc.sbuf_tensor([P, 2, S, AW, 2], bf16))
        t1 = ctx.enter_context(nc.sbuf_tensor([P, 2, S * AW], bf16))
        colredA = ctx.enter_context(nc.sbuf_tensor([P, K, S, AW], bf16))
        redA = ctx.enter_context(nc.sbuf_tensor([P, K, S, AW], bf16))
        y4 = ctx.enter_context(nc.sbuf_tensor([P, K, S, 4], bf16))
        y2 = ctx.enter_context(nc.sbuf_tensor([P, K, S, 2], bf16))
        y1 = ctx.enter_context(nc.sbuf_tensor([P, K, S, 1], bf16))
        res = ctx.enter_context(nc.sbuf_tensor([P, K, S], bf16))
        psA = ctx.enter_context(nc.psum_tensor([2, 4096], f32))
        sIN = ctx.enter_context(nc.semaphore("sIN"))    # delta2
        sI2 = ctx.enter_context(nc.semaphore("sI2"))    # ones
        sOF = ctx.enter_context(nc.semaphore("sOF"))    # offsets
        sC = ctx.enter_context(nc.semaphore("sC"))      # impad writes
        sD = ctx.enter_context(nc.semaphore("sD"))      # strip quarters
        sO = ctx.enter_context(nc.semaphore("sO"))      # out
        sX = [ctx.enter_context(nc.semaphore(f"sX{q}"))
              for q in range(len(XCHUNKS))]
        pe = ctx.enter_context(nc.semaphore("pe"))      # matmuls
        actC = ctx.enter_context(nc.semaphore("actC"))  # mean copies
        actX = ctx.enter_context(nc.semaphore("actX"))  # wX groups
        actY = ctx.enter_context(nc.semaphore("actY"))  # wY done
        dve = ctx.enter_context(nc.semaphore("dve"))
        block = ctx.enter_context(nc.Block())

        # dve milestones: 1 memset, 2 ox_cl, 3 oy_cl, 4/5/6 dX groups,
        # 7 first 5 taps reduced, 8 all taps reduced
        DVE_FINA, DVE_FIN = 7, 8

        @block.sync
        def _(sync):
            sync.dma_start(out=delta_sb[:].rearrange("p a b c -> p (a b c)"),
                           in_=delta_ext[:]).then_inc(sIN, 16)
            sync.dma_start(out=ones_sb[:], in_=ones_ext[:]).then_inc(sI2, 16)
            sync.dma_start(out=biasy_sb[:], in_=biasy_ext[:]).then_inc(sIN, 16)
            c0 = 0
            for q, n in enumerate(XCHUNKS):
                if QMAP[q] == 0:
                    sync.dma_start(
                        out=xb[:, c0 * NCH:(c0 + n) * NCH],
                        in_=x_ext[:, c0 * NCH:(c0 + n) * NCH]).then_inc(sX[q], 16)
                c0 += n
            sync.wait_ge(dve, 1)
            sync.dma_start(
                out=bass.AP(tensor=impad[:].tensor, offset=impad[:].offset,
                            ap=[[1, 1], [1, PADR * W]]),
                in_=zt[:]).then_inc(sC, 16)
            sync.dma_start(
                out=bass.AP(tensor=impad[:].tensor,
                            offset=impad[:].offset + NPAD - PADR * W,
                            ap=[[1, 1], [1, PADR * W]]),
                in_=zt[:]).then_inc(sC, 16)
            sixth = HW // 6
            for s6 in range(6):
                sync.wait_ge(actC, SIXTH_GATE[s6])
                sync.dma_start(
                    out=bass.AP(tensor=impad[:].tensor,
                                offset=impad[:].offset + PADR * W + s6 * sixth,
                                ap=[[1, 1], [1, sixth]]),
                    in_=m_flat[:, s6 * sixth:(s6 + 1) * sixth]).then_inc(sC, 16)
            sync.wait_ge(sC, STRIP_GATE[0])
            sync.dma_start(
                out=strip[0:32],
                in_=bass.AP(tensor=impad[:].tensor,
                            offset=impad[:].offset + STRIPOFF,
                            ap=[[S, 32], [1, STRIPLEN]])).then_inc(sD, 16)
            sync.wait_ge(sC, STRIP_GATE[2])
            sync.dma_start(
                out=strip[64:96],
                in_=bass.AP(tensor=impad[:].tensor,
                            offset=impad[:].offset + STRIPOFF + 64 * S,
                            ap=[[S, 32], [1, STRIPLEN]])).then_inc(sD, 16)
            sync.wait_ge(sC, STRIP_GATE[3])
            sync.dma_start(
                out=strip[96:128],
                in_=bass.AP(tensor=impad[:].tensor,
                            offset=impad[:].offset + STRIPOFF + 96 * S,
                            ap=[[S, 32], [1, STRIPLEN]])).then_inc(sD, 16)
            sync.wait_ge(dve, DVE_FINA)
            sync.dma_start(
                out=out_ext[:, 0:5 * S],
                in_=res[:, 0:5].rearrange("p k s -> p (k s)")).then_inc(sO, 16)
            sync.wait_ge(dve, DVE_FIN)
            sync.dma_start(
                out=out_ext[:, 5 * S:],
                in_=res[:, 5:9].rearrange("p k s -> p (k s)")).then_inc(sO, 16)

        @block.gpsimd
        def _(g):
            g.dma_start(
                out=off_sb[:].rearrange("p a b -> p (a b)"),
                in_=off_ext[:]).then_inc(sOF, 16)
            c0 = 0
            for q, n in enumerate(XCHUNKS):
                if QMAP[q] == 2:
                    g.dma_start(
                        out=xb[:, c0 * NCH:(c0 + n) * NCH],
                        in_=x_ext[:, c0 * NCH:(c0 + n) * NCH]).then_inc(sX[q], 16)
                c0 += n
            g.wait_ge(sC, STRIP_GATE[1])
            g.dma_start(
                out=strip[32:64],
                in_=bass.AP(tensor=impad[:].tensor,
                            offset=impad[:].offset + STRIPOFF + 32 * S,
                            ap=[[S, 32], [1, STRIPLEN]])).then_inc(sD, 16)

        @block.tensor
        def _(tensor):
            tensor.wait_ge(sI2, 16)   # ones
            g = 0
            for q, n in enumerate(XCHUNKS):
                tensor.wait_ge(sX[q], 16)
                for _ in range(n):
                    if g >= 8:
                        tensor.wait_ge(actC, (g - 8) // 2 + 1)
                    nc.tensor.matmul(
                        psA[:, (g % 8) * NCH:(g % 8 + 1) * NCH],
                        ones_sb[:],
                        xb[:, g * NCH:(g + 1) * NCH],
                        start=True, stop=True,
                    ).then_inc(pe, 1)
                    g += 1


        @block.scalar
        def _(scalar):
            c0 = 0
            for q, n in enumerate(XCHUNKS):
                if QMAP[q] == 1:
                    scalar.dma_start(
                        out=xb[:, c0 * NCH:(c0 + n) * NCH],
                        in_=x_ext[:, c0 * NCH:(c0 + n) * NCH]).then_inc(sX[q], 16)
                c0 += n

            def copy(c):
                scalar.wait_ge(pe, 2 * c + 2)
                nc.scalar.activation(
                    m_flat[:, c * 1024:(c + 1) * 1024],
                    psA[0:1, (2 * c % 8) * NCH:(2 * c % 8 + 2) * NCH],
                    Act.Copy,
                ).then_inc(actC, 1)

            def xgroup(kx):
                scalar.wait_ge(dve, 4 + kx)
                nc.scalar.activation(aXt[:, kx], dXt[:, kx], Act.Abs)
                # wX = relu(1 - |dX|), written transposed to [t, s, j]
                wx_out = bass.AP(
                    tensor=wX[:].tensor,
                    offset=wX[:].offset + kx * S * AW,
                    ap=[list(wX[:].ap[0])]
                    + [[3 * S * AW, 3], [1, AW], [AW, S]])
                nc.scalar.activation(wx_out, aXt[:, kx], Act.Relu,
                                     bias=1.0, scale=-1.0).then_inc(actX, 1)

            for c in range(5):
                copy(c)
            xgroup(0)
            for c in range(5, 9):
                copy(c)
            xgroup(1)
            xgroup(2)
            scalar.wait_ge(dve, 3)
            scalar.wait_ge(sIN, 32)
            for j in range(AW):
                nc.scalar.activation(absY[:, :, :, j:j + 1],
                                     oy_cl[:].unsqueeze(3), Act.Abs,
                                     bias=biasy_sb[:, j:j + 1])
            nc.scalar.activation(
                wY[:].rearrange("p a b c -> p (a b c)"),
                absY[:].rearrange("p a b c -> p (a b c)"),
                Act.Relu, bias=1.0, scale=-1.0).then_inc(actY, 1)

        @block.vector
        def _(vector):
            nc.vector.memset(zt[:], 0.0).then_inc(dve, 1)
            vector.wait_ge(sOF, 16)
            # batched clamps over all taps (x maps odd j, y maps even j)
            nc.vector.tensor_scalar(
                ox_cl[:], bass.AP(tensor=off_sb[:].tensor,
                                  offset=off_sb[:].offset + S,
                                  ap=[list(off_sb[:].ap[0])] + [[2 * S, K],
                                                                [1, S]]),
                CLAMP, -CLAMP, Alu.min, Alu.max).then_inc(dve, 1)
            nc.vector.tensor_scalar(
                oy_cl[:], bass.AP(tensor=off_sb[:].tensor,
                                  offset=off_sb[:].offset,
                                  ap=[list(off_sb[:].ap[0])] + [[2 * S, K],
                                                                [1, S]]),
                CLAMP, -CLAMP, Alu.min, Alu.max).then_inc(dve, 1)
            vector.wait_ge(sIN, 16)
            for kx in range(3):
                # dXt[p, kx, t, j, s] = ox_cl[p, kx+3t, s] + delta2[p, kx, j, s]
                oxg = bass.AP(tensor=ox_cl[:].tensor,
                              offset=ox_cl[:].offset + kx * S,
                              ap=[list(ox_cl[:].ap[0])] + [[3 * S, 3],
                                                           [0, AW], [1, S]])
                dlt = bass.AP(tensor=delta_sb[:].tensor,
                              offset=delta_sb[:].offset + kx * AW * S,
                              ap=[list(delta_sb[:].ap[0])] + [[0, 3],
                                                              [S, AW], [1, S]])
                nc.vector.tensor_tensor(dXt[:, kx], oxg, dlt,
                                        Alu.add).then_inc(dve, 1)
            for k in range(K):
                ky = k // 3
                kx = k % 3
                s = k % 2
                if k == 0:
                    vector.wait_ge(sD, 64)
                vector.wait_ge(actX, kx + 1)
                wxb = wX[:, k].unsqueeze(2).broadcast_to([P, S, AW, AW])
                ska = bass.AP(tensor=strip[:].tensor,
                              offset=strip[:].offset + 96 * ky + kx,
                              ap=[list(strip[:].ap[0])] + [[1, S], [96, AW],
                                                           [1, AW]])
                nc.vector.tensor_tensor(prod[:, s], wxb, ska, Alu.mult)
                nc.vector.tensor_add(t4[:, s], prod[:, s, :, :, 0:4],
                                     prod[:, s, :, :, 4:8])
                nc.vector.tensor_add(t2[:, s], t4[:, s, :, :, 0:2],
                                     t4[:, s, :, :, 2:4])
                t2base = t2[:].offset + s * S * AW * 2
                nc.vector.tensor_add(
                    t1[:, s],
                    bass.AP(tensor=t2[:].tensor, offset=t2base,
                            ap=[list(t2[:].ap[0])] + [[2, S * AW]]),
                    bass.AP(tensor=t2[:].tensor, offset=t2base + 1,
                            ap=[list(t2[:].ap[0])] + [[2, S * AW]]))
                nc.vector.tensor_add(
                    bass.AP(tensor=colredA[:].tensor,
                            offset=colredA[:].offset + k * S * AW,
                            ap=[list(colredA[:].ap[0])] + [[1, S * AW]]),
                    t1[:, s],
                    bass.AP(tensor=prod[:].tensor,
                            offset=prod[:].offset + s * S * AW * AW + 8,
                            ap=[list(prod[:].ap[0])] + [[AW, S * AW]]))
                if k in (4, 8):
                    lo, hi = (0, 5) if k == 4 else (5, 9)
                    vector.wait_ge(actY, 1)
                    nc.vector.tensor_mul(redA[:, lo:hi], colredA[:, lo:hi],
                                         wY[:, lo:hi])
                    nc.vector.tensor_add(y4[:, lo:hi],
                                         redA[:, lo:hi, :, 0:4],
                                         redA[:, lo:hi, :, 4:8])
                    nc.vector.tensor_add(y2[:, lo:hi],
                                         y4[:, lo:hi, :, 0:2],
                                         y4[:, lo:hi, :, 2:4])
                    nc.vector.tensor_add(y1[:, lo:hi],
                                         y2[:, lo:hi, :, 0:1],
                                         y2[:, lo:hi, :, 1:2])
                    nc.vector.tensor_add(res[:, lo:hi],
                                         y1[:, lo:hi, :, 0],
                                         redA[:, lo:hi, :, 8]).then_inc(dve, 1)

    return nc


def _bf16_dtype():
    import ml_dtypes
    return ml_dtypes.bfloat16


def _tables():
    import ml_dtypes
    p = np.arange(P)[:, None, None, None]
    kx = np.arange(3)[None, :, None, None]
    j = np.arange(AW)[None, None, :, None]
    s = np.arange(S)[None, None, None, :]
    u = (S * p + s) % 96
    v = ((S * p + kx + s + j - 5) % 96) - (kx - 1)
    delta2 = (u - v).astype(np.float16).reshape(P, 3 * AW * S)
    ones = np.full((C, 2), 1.0 / C, dtype=ml_dtypes.bfloat16)
    biasy = np.tile(4.0 - np.arange(AW, dtype=np.float32), (P, 1))
    return delta2, ones, biasy


def _get_nc():
    if "nc" not in _cached:
        _cached["nc"] = _build_nc()
    return _cached["nc"]


def _run(x, offset, trace=False):
    from concourse.bass_utils import run_bass_kernel_spmd

    nc = _get_nc()
    delta2, ones, biasy = _tables()

    in_maps = []
    for b in range(B):
        in_maps.append({
            "x": np.ascontiguousarray(x[b].reshape(C, HW)).astype(
                _bf16_dtype()),
            "offset": np.ascontiguousarray(
                offset[b].reshape(2 * K, P, S).swapaxes(0, 1)
                .reshape(P, 2 * K * S)).astype(np.float16),
            "delta2": delta2,
            "ones": ones,
            "biasy": biasy,
        })

    return run_bass_kernel_spmd(nc, in_maps, list(range(B)), trace=trace)


def kernel(x: np.ndarray, offset: np.ndarray, weight: np.ndarray) -> np.ndarray:
    results = _run(x, offset).results

    # host epilogue: replicate over t with per-(t,k) channel-sum scaling
    s = weight.reshape(C, T * K).sum(axis=0).astype(np.float32)  # [T*K]
    out = np.empty((B, T * K, H, W), dtype=np.float32)
    for b in range(B):
        samp = (results[b]["out"].astype(np.float32)
                .reshape(P, K, S).transpose(1, 0, 2).reshape(K, H, W))
        for t in range(T):
            out[b, t * K:(t + 1) * K] = s[t * K:(t + 1) * K, None, None] * samp
    return out


# revision 10
# speedup vs baseline: 1.2303x; 1.2303x over previous
"""Deformable-correlation-fixed-weight kernel for 8 TRN2 NeuronCores.

Math: out[b, t*K+k, h, w] = sum_c samp[b,c,k,h,w] * weight[c,t,k].
With weight constant along c (DefCorFixW: weight = 1/C), this equals
s[t,k] * bilinear(mean_c x[b], py[b,k], px[b,k]); the device computes
the channel-mean image and the 9 bilinear-sampled maps per batch; the
host replicates over t and scales by s[t,k] = sum_c weight[c,t,k].

Sharding: data-parallel over batch B=8 across the 8 cores.

v4 design (triple-fused taps):
  Pixels n = h*96+w are laid out n = 72*p + i (p partition, i slot).
  Offsets clamped to +-3.9995 => a 9x9 hat window suffices.

  The 9 taps k = 3*ky + kx group into 3 triples of constant ky whose
  strip windows differ only by kx = 0,1,2 -> adjacent columns. All
  window/tree/y ops process one triple with tap t as a packed
  innermost dim of size 3, so EVERY level of the reduction tree has a
  unit-stride innermost run (2x DVE mode), including the former
  1x tail adds. Weight tensors live in [s, j, t] layouts; |.| and
  relu(1-.) run on ACT over contiguous views.

  Ramp: mean-matmul chain chases 512-elem x DMA chunks; ACT drains
  PSUM in 9 1024-wide copies (1/C folded into `ones`); impad written
  in 6 sixths; strip fetched in 4 gated quarter DMAs. Output in bf16.
"""

import numpy as np

B, C, H, W = 8, 128, 96, 96
K = 9
T = 9
HW = H * W
P = 128          # partitions
S = HW // P      # 72 pixels per partition
AW = 9           # window side (rows and cols)
CLAMP = 3.9995
PADR = 8         # zero rows above/below in the flat padded image
NPAD = (H + 2 * PADR) * W          # 10752
STRIPLEN = 1042                    # per-partition strip (flat span)
STRIPOFF = 283                     # 72p - 485 + PADR*96
NCH = 512
NCHUNK = HW // NCH                 # 18
XCHUNKS = (1, 2, 3, 3, 3, 3, 3)
QMAP = (0, 1, 2, 1, 2, 1, 0)
# impad sixth s may be written after ACT copy SIXTH_GATE[s] (1-based)
SIXTH_GATE = (2, 3, 5, 6, 8, 9)
# strip quarter q needs sC >= STRIP_GATE[q] (zt,zb,s0..s5 in order, 16 each)
STRIP_GATE = (64, 96, 112, 128)

_cached = {}


def _build_nc():
    import concourse.bass as bass
    import concourse.mybir as mybir
    from contextlib import ExitStack

    f32 = mybir.dt.float32
    bf16 = mybir.dt.bfloat16
    fp16 = mybir.dt.float16
    Alu = mybir.AluOpType
    Act = mybir.ActivationFunctionType

    nc = bass.Bass(detect_race_conditions=False)

    x_ext = nc.declare_dram_parameter("x", [C, HW], bf16, isOutput=False)
    off_ext = nc.declare_dram_parameter("offset", [P, 2 * K * S], fp16,
                                        isOutput=False)
    delta_ext = nc.declare_dram_parameter("delta3", [P, 3 * S * AW], fp16,
                                          isOutput=False)
    iotay_ext = nc.declare_dram_parameter("iotay", [P, AW], fp16,
                                          isOutput=False)
    ones_ext = nc.declare_dram_parameter("ones", [C, 2], bf16, isOutput=False)
    out_ext = nc.declare_dram_parameter("out", [P, K * S], bf16, isOutput=True)

    impad = nc.dram_tensor("impad", [NPAD], bf16)

    with ExitStack() as ctx:
        xb = ctx.enter_context(nc.sbuf_tensor([C, HW], bf16))
        off_sb = ctx.enter_context(nc.sbuf_tensor([P, 2 * K, S], fp16))
        delta_sb = ctx.enter_context(nc.sbuf_tensor([P, 3, S, AW], fp16))
        iotay_sb = ctx.enter_context(nc.sbuf_tensor([P, AW], fp16))
        ones_sb = ctx.enter_context(nc.sbuf_tensor([C, 2], bf16))
        m_flat = ctx.enter_context(nc.sbuf_tensor([1, HW], bf16))
        zt = ctx.enter_context(nc.sbuf_tensor([1, PADR * W], bf16))
        strip = ctx.enter_context(nc.sbuf_tensor([P, STRIPLEN], bf16))
        ox_cl = ctx.enter_context(nc.sbuf_tensor([P, K, S], fp16))
        oy_cl = ctx.enter_context(nc.sbuf_tensor([P, K, S], fp16))
        dX3 = ctx.enter_context(nc.sbuf_tensor([P, 3, S, AW, 3], fp16))
        dY3 = ctx.enter_context(nc.sbuf_tensor([P, 3, S, AW, 3], fp16))
        aX3 = ctx.enter_context(nc.sbuf_tensor([P, S * AW * 3], fp16))
        aY3 = ctx.enter_context(nc.sbuf_tensor([P, S * AW * 3], fp16))
        wX3 = ctx.enter_context(nc.sbuf_tensor([P, 3, S, AW, 3], bf16))
        wY3 = ctx.enter_context(nc.sbuf_tensor([P, 3, S, AW, 3], bf16))
        prod = ctx.enter_context(nc.sbuf_tensor([P, S, AW, AW, 3], bf16))
        t4 = ctx.enter_context(nc.sbuf_tensor([P, S, AW, 4, 3], bf16))
        t2 = ctx.enter_context(nc.sbuf_tensor([P, S, AW, 2, 3], bf16))
        t1 = ctx.enter_context(nc.sbuf_tensor([P, S * AW * 3], bf16))
        colredA = ctx.enter_context(nc.sbuf_tensor([P, 3, S * AW * 3], bf16))
        redA = ctx.enter_context(nc.sbuf_tensor([P, S, AW, 3], bf16))
        y4 = ctx.enter_context(nc.sbuf_tensor([P, S, 4, 3], bf16))
        y2 = ctx.enter_context(nc.sbuf_tensor([P, S, 2, 3], bf16))
        y1 = ctx.enter_context(nc.sbuf_tensor([P, S * 3], bf16))
        res = ctx.enter_context(nc.sbuf_tensor([P, 3, S * 3], bf16))
        psA = ctx.enter_context(nc.psum_tensor([2, 4096], f32))
        sIN = ctx.enter_context(nc.semaphore("sIN"))    # delta3 + iotay
        sI2 = ctx.enter_context(nc.semaphore("sI2"))    # ones
        sOF = ctx.enter_context(nc.semaphore("sOF"))    # offsets
        sC = ctx.enter_context(nc.semaphore("sC"))      # impad writes
        sD = ctx.enter_context(nc.semaphore("sD"))      # strip quarters
        sO = ctx.enter_context(nc.semaphore("sO"))      # out
        sX = [ctx.enter_context(nc.semaphore(f"sX{q}"))
              for q in range(len(XCHUNKS))]
        pe = ctx.enter_context(nc.semaphore("pe"))      # matmuls
        actC = ctx.enter_context(nc.semaphore("actC"))  # mean copies
        actX = ctx.enter_context(nc.semaphore("actX"))  # wX3 triples
        actY = ctx.enter_context(nc.semaphore("actY"))  # wY3 triples
        dve = ctx.enter_context(nc.semaphore("dve"))
        block = ctx.enter_context(nc.Block())

        # dve milestones: 1 memset, 2 ox, 3 oy, 4 dX0, 5 dY0, 6 dX1,
        # 7 dY1, 8 dX2, 9 dY2, 10 y0, 11 y1, 12 y2
        def pap(t_ap, off, dims):
            return bass.AP(tensor=t_ap.tensor, offset=t_ap.offset + off,
                           ap=[list(t_ap.ap[0])] + dims)

        @block.sync
        def _(sync):
            c0 = 0
            for q, n in enumerate(XCHUNKS):
                if QMAP[q] == 0 and q == 0:
                    sync.dma_start(
                        out=xb[:, c0 * NCH:(c0 + n) * NCH],
                        in_=x_ext[:, c0 * NCH:(c0 + n) * NCH]).then_inc(sX[q], 16)
                c0 += n
            sync.dma_start(out=delta_sb[:].rearrange("p a b c -> p (a b c)"),
                           in_=delta_ext[:]).then_inc(sIN, 16)
            sync.dma_start(out=iotay_sb[:], in_=iotay_ext[:]).then_inc(sIN, 16)
            sync.dma_start(out=ones_sb[:], in_=ones_ext[:]).then_inc(sI2, 16)
            c0 = 0
            for q, n in enumerate(XCHUNKS):
                if QMAP[q] == 0 and q > 0:
                    sync.dma_start(
                        out=xb[:, c0 * NCH:(c0 + n) * NCH],
                        in_=x_ext[:, c0 * NCH:(c0 + n) * NCH]).then_inc(sX[q], 16)
                c0 += n
            sync.wait_ge(dve, 1)
            sync.dma_start(
                out=bass.AP(tensor=impad[:].tensor, offset=impad[:].offset,
                            ap=[[1, 1], [1, PADR * W]]),
                in_=zt[:]).then_inc(sC, 16)
            sync.dma_start(
                out=bass.AP(tensor=impad[:].tensor,
                            offset=impad[:].offset + NPAD - PADR * W,
                            ap=[[1, 1], [1, PADR * W]]),
                in_=zt[:]).then_inc(sC, 16)
            sixth = HW // 6
            for s6 in range(6):
                sync.wait_ge(actC, SIXTH_GATE[s6])
                sync.dma_start(
                    out=bass.AP(tensor=impad[:].tensor,
                                offset=impad[:].offset + PADR * W + s6 * sixth,
                                ap=[[1, 1], [1, sixth]]),
                    in_=m_flat[:, s6 * sixth:(s6 + 1) * sixth]).then_inc(sC, 16)
            for q, gate in ((0, STRIP_GATE[0]), (2, STRIP_GATE[2]),
                            (3, STRIP_GATE[3])):
                sync.wait_ge(sC, gate)
                sync.dma_start(
                    out=strip[32 * q:32 * (q + 1)],
                    in_=bass.AP(tensor=impad[:].tensor,
                                offset=impad[:].offset + STRIPOFF + 32 * q * S,
                                ap=[[S, 32], [1, STRIPLEN]])).then_inc(sD, 16)
            sync.wait_ge(dve, 11)
            sync.dma_start(
                out=out_ext[:, 0:6 * S],
                in_=res[:, 0:2].rearrange("p a b -> p (a b)")).then_inc(sO, 16)
            sync.wait_ge(dve, 12)
            sync.dma_start(
                out=out_ext[:, 6 * S:],
                in_=res[:, 2]).then_inc(sO, 16)

        @block.gpsimd
        def _(g):
            g.dma_start(
                out=off_sb[:].rearrange("p a b -> p (a b)"),
                in_=off_ext[:]).then_inc(sOF, 16)
            c0 = 0
            for q, n in enumerate(XCHUNKS):
                if QMAP[q] == 2:
                    g.dma_start(
                        out=xb[:, c0 * NCH:(c0 + n) * NCH],
                        in_=x_ext[:, c0 * NCH:(c0 + n) * NCH]).then_inc(sX[q], 16)
                c0 += n
            g.wait_ge(sC, STRIP_GATE[1])
            g.dma_start(
                out=strip[32:64],
                in_=bass.AP(tensor=impad[:].tensor,
                            offset=impad[:].offset + STRIPOFF + 32 * S,
                            ap=[[S, 32], [1, STRIPLEN]])).then_inc(sD, 16)

        @block.tensor
        def _(tensor):
            tensor.wait_ge(sI2, 16)   # ones
            g = 0
            for q, n in enumerate(XCHUNKS):
                tensor.wait_ge(sX[q], 16)
                for _ in range(n):
                    if g >= 8:
                        tensor.wait_ge(actC, (g - 8) // 2 + 1)
                    nc.tensor.matmul(
                        psA[:, (g % 8) * NCH:(g % 8 + 1) * NCH],
                        ones_sb[:],
                        xb[:, g * NCH:(g + 1) * NCH],
                        start=True, stop=True,
                    ).then_inc(pe, 1)
                    g += 1

        @block.scalar
        def _(scalar):
            c0 = 0
            for q, n in enumerate(XCHUNKS):
                if QMAP[q] == 1:
                    scalar.dma_start(
                        out=xb[:, c0 * NCH:(c0 + n) * NCH],
                        in_=x_ext[:, c0 * NCH:(c0 + n) * NCH]).then_inc(sX[q], 16)
                c0 += n
            for c in range(9):
                scalar.wait_ge(pe, 2 * c + 2)
                nc.scalar.activation(
                    m_flat[:, c * 1024:(c + 1) * 1024],
                    psA[0:1, (2 * c % 8) * NCH:(2 * c % 8 + 2) * NCH],
                    Act.Copy,
                ).then_inc(actC, 1)
            for tr in range(3):
                scalar.wait_ge(dve, 4 + 2 * tr)
                nc.scalar.activation(
                    aX3[:], dX3[:, tr].rearrange("p a b c -> p (a b c)"),
                    Act.Abs)
                nc.scalar.activation(
                    wX3[:, tr].rearrange("p a b c -> p (a b c)"), aX3[:],
                    Act.Relu, bias=1.0, scale=-1.0).then_inc(actX, 1)
                scalar.wait_ge(dve, 5 + 2 * tr)
                nc.scalar.activation(
                    aY3[:], dY3[:, tr].rearrange("p a b c -> p (a b c)"),
                    Act.Abs)
                nc.scalar.activation(
                    wY3[:, tr].rearrange("p a b c -> p (a b c)"), aY3[:],
                    Act.Relu, bias=1.0, scale=-1.0).then_inc(actY, 1)

        @block.vector
        def _(vector):
            nc.vector.memset(zt[:], 0.0).then_inc(dve, 1)
            vector.wait_ge(sOF, 16)
            # batched clamps over all taps (x maps odd j, y maps even j)
            nc.vector.tensor_scalar(
                ox_cl[:], pap(off_sb[:], S, [[2 * S, K], [1, S]]),
                CLAMP, -CLAMP, Alu.min, Alu.max).then_inc(dve, 1)
            nc.vector.tensor_scalar(
                oy_cl[:], pap(off_sb[:], 0, [[2 * S, K], [1, S]]),
                CLAMP, -CLAMP, Alu.min, Alu.max).then_inc(dve, 1)
            vector.wait_ge(sIN, 32)
            for tr in range(3):
                # dX3[p,tr,s,j,t] = ox_cl[p,3tr+t,s] + delta3[p,t,s,j]
                nc.vector.tensor_tensor(
                    dX3[:, tr],
                    pap(ox_cl[:], 3 * tr * S, [[1, S], [0, AW], [S, 3]]),
                    pap(delta_sb[:], 0, [[AW, S], [1, AW], [S * AW, 3]]),
                    Alu.add).then_inc(dve, 1)
                # dY3[p,tr,s,j,t] = oy_cl[p,3tr+t,s] - (j - 4)
                nc.vector.tensor_tensor(
                    dY3[:, tr],
                    pap(oy_cl[:], 3 * tr * S, [[1, S], [0, AW], [S, 3]]),
                    pap(iotay_sb[:], 0, [[0, S], [1, AW], [0, 3]]),
                    Alu.subtract).then_inc(dve, 1)
            for tr in range(3):
                if tr == 0:
                    vector.wait_ge(sD, 64)
                vector.wait_ge(actX, tr + 1)
                # prod[p,s,dy,dx,t] = wX3[p,tr,s,dx,t] * strip[p, s+96*tr
                #   +96*dy+dx+t]; one op per dy to stay within 3 free dims
                for dy in range(AW):
                    nc.vector.tensor_tensor(
                        pap(prod[:], 27 * dy, [[243, S], [1, 27]]),
                        pap(wX3[:], tr * S * AW * 3, [[AW * 3, S], [1, 27]]),
                        pap(strip[:], 96 * tr + 96 * dy,
                            [[1, S], [1, AW], [1, 3]]),
                        Alu.mult)
                nc.vector.tensor_add(
                    pap(t4[:], 0, [[108, S], [12, AW], [1, 12]]),
                    pap(prod[:], 0, [[243, S], [27, AW], [1, 12]]),
                    pap(prod[:], 12, [[243, S], [27, AW], [1, 12]]))
                nc.vector.tensor_add(
                    pap(t2[:], 0, [[54, S], [6, AW], [1, 6]]),
                    pap(t4[:], 0, [[108, S], [12, AW], [1, 6]]),
                    pap(t4[:], 6, [[108, S], [12, AW], [1, 6]]))
                nc.vector.tensor_add(
                    t1[:],
                    pap(t2[:], 0, [[54, S], [6, AW], [1, 3]]),
                    pap(t2[:], 3, [[54, S], [6, AW], [1, 3]]))
                nc.vector.tensor_add(
                    colredA[:, tr], t1[:],
                    pap(prod[:], 24, [[243, S], [27, AW], [1, 3]]))
                vector.wait_ge(actY, tr + 1)
                nc.vector.tensor_mul(
                    redA[:].rearrange("p a b c -> p (a b c)"),
                    colredA[:, tr],
                    wY3[:, tr].rearrange("p a b c -> p (a b c)"))
                nc.vector.tensor_add(
                    y4[:],
                    pap(redA[:], 0, [[27, S], [3, 4], [1, 3]]),
                    pap(redA[:], 12, [[27, S], [3, 4], [1, 3]]))
                nc.vector.tensor_add(
                    y2[:],
                    pap(y4[:], 0, [[12, S], [3, 2], [1, 3]]),
                    pap(y4[:], 6, [[12, S], [3, 2], [1, 3]]))
                nc.vector.tensor_add(
                    y1[:],
                    pap(y2[:], 0, [[6, S], [1, 3]]),
                    pap(y2[:], 3, [[6, S], [1, 3]]))
                nc.vector.tensor_add(
                    res[:, tr], y1[:],
                    pap(redA[:], 24, [[27, S], [1, 3]])).then_inc(dve, 1)

    return nc


def _bf16_dtype():
    import ml_dtypes
    return ml_dtypes.bfloat16


def _tables():
    import ml_dtypes
    p = np.arange(P)[:, None, None, None]
    kx = np.arange(3)[None, :, None, None]
    s = np.arange(S)[None, None, :, None]
    j = np.arange(AW)[None, None, None, :]
    u = (S * p + s) % 96
    v = ((S * p + kx + s + j - 5) % 96) - (kx - 1)
    delta3 = (u - v).astype(np.float16).reshape(P, 3 * S * AW)
    iotay = np.tile(np.arange(AW, dtype=np.float16) - 4.0, (P, 1))
    ones = np.full((C, 2), 1.0 / C, dtype=ml_dtypes.bfloat16)
    return delta3, iotay, ones


def _get_nc():
    if "nc" not in _cached:
        _cached["nc"] = _build_nc()
    return _cached["nc"]


def _run(x, offset, trace=False):
    from concourse.bass_utils import run_bass_kernel_spmd

    nc = _get_nc()
    delta3, iotay, ones = _tables()

    in_maps = []
    for b in range(B):
        in_maps.append({
            "x": np.ascontiguousarray(x[b].reshape(C, HW)).astype(
                _bf16_dtype()),
            "offset": np.ascontiguousarray(
                offset[b].reshape(2 * K, P, S).swapaxes(0, 1)
                .reshape(P, 2 * K * S)).astype(np.float16),
            "delta3": delta3,
            "iotay": iotay,
            "ones": ones,
        })

    return run_bass_kernel_spmd(nc, in_maps, list(range(B)), trace=trace)


def kernel(x: np.ndarray, offset: np.ndarray, weight: np.ndarray) -> np.ndarray:
    results = _run(x, offset).results

    # host epilogue: replicate over t with per-(t,k) channel-sum scaling
    s = weight.reshape(C, T * K).sum(axis=0).astype(np.float32)  # [T*K]
    out = np.empty((B, T * K, H, W), dtype=np.float32)
    for b in range(B):
        # device layout: [P, tr, S, t] with k = 3*tr + t
        samp = (results[b]["out"].astype(np.float32)
                .reshape(P, 3, S, 3).transpose(1, 3, 0, 2)
                .reshape(K, H, W))
        for t in range(T):
            out[b, t * K:(t + 1) * K] = s[t * K:(t + 1) * K, None, None] * samp
    return out
